# revision 7
# baseline (speedup 1.0000x reference)
"""Pairwise cosine-similarity (normalize -> x @ x.T) + Linear(1,2) affine, on 8 trn2 cores.

Data-parallel over rows of x; each core computes a [512, 4096] slice of the
similarity matrix and writes [512, 4096, 2] fp32 output.

v2 design (vs the first working version):
  - Host-side rotation: core c receives x rolled so its own 512 rows come
    FIRST.  The kernel is pure SPMD with compile-time offsets (no per-core
    branching), stationary operand = xnT block 0.  The host un-rotates the
    output columns (np.roll on axis 1) after the run.
  - fp32 -> fp16 cast during the input DMA (SWDGE): halves SBUF-side input
    traffic and doubles effective vector-engine throughput for the whole
    prep pipeline (sumsq, normalize, transpose copies all fp16).
  - Row norms: fp16 squares accumulated in fp32 (accum_out), batched
    reciprocal+sqrt per 4-tile group, normalize fused with per-row scale.
  - PE transposes (fp16, 128x128) into 1-bank PSUM tiles; batched
    PSUM->SBUF copies build xnT [128, 6, 4096] fp16.
  - Matmul units: [128, 1024] fp32 PSUM (2 banks) = 12 fp16 matmuls
    (2 column blocks x 6 k-tiles).  Epilogue: 2 wide ops per unit
    (ACT ch0 / DVE ch1) writing interleaved fp16 [128, 1024, 2]; the
    output DMA casts fp16 -> fp32 in the SWDGE datapath.
  - Warm-up matmuls keep the PE HAM clock at 2.4 GHz through the prep phase.

Numerics: fp16 inputs/outputs with fp32 accumulation everywhere
(PE PSUM + activation internals).  Measured rel err ~1.5e-4 vs the fp32
reference (tolerance 2e-2).

This file monkeypatches two toolchain gaps at import: walrus here only
accepts one sync-wait per instruction (Tile emits several), and the
axon NTFF profile hook module may be absent when BASS_TRACE=1.
"""

import numpy as np
from contextlib import ExitStack

import concourse.bass as bass
import concourse.tile as tile
from concourse import mybir
from concourse.bass_utils import run_bass_kernel_spmd

B, D, NCORES = 4096, 768, 8
BC = B // NCORES          # 512 rows per core
P = 128                   # partitions
KT = D // P               # 6 contraction tiles
NB = B // 512             # 8 column blocks of 512
F16 = mybir.dt.float16
F32 = mybir.dt.float32
AF = mybir.ActivationFunctionType
ALU = mybir.AluOpType

LAST_RESULTS = None       # test harness peeks at exec_time_ns here


def _legalize_single_wait(bir_bytes: bytes) -> bytes:
    """This container's walrus accepts at most ONE sync wait per instruction,
    while Tile attaches several. Split extras into standalone EventSemaphore
    instructions inserted just before the owner (same engine stream, so the
    sequencer stalls at the same program point; schedule order is a global
    topological order, so earlier stalls cannot deadlock)."""
    import json

    d = json.loads(bir_bytes)
    n_split = 0
    for f in d.get("functions", []):
        for bb in f.get("blocks", []):
            insts = bb.get("instructions", [])
            out = []
            for ins in insts:
                si = ins.get("sync_info") or {}
                waits = si.get("on_wait") or []
                if len(waits) > 1:
                    keep = waits[-1]
                    for i, w in enumerate(waits[:-1]):
                        n_split += 1
                        out.append({
                            "debug": ins.get("debug", 0),
                            "engine": ins["engine"],
                            "ins": [],
                            "name": f"{ins['name']}__w{i}",
                            "opcode": "EventSemaphore",
                            "outs": [],
                            "sync_info": {"on_update": [], "on_wait": [w]},
                        })
                    si["on_wait"] = [keep]
                out.append(ins)
            bb["instructions"] = out
    return json.dumps(d).encode()


def _install_walrus_shim():
    """Route every BIR->NEFF compile through the single-wait legalizer."""
    import concourse.bass2jax as b2j
    import concourse.bass_utils as bu

    if getattr(bu, "_single_wait_shim", False):
        return
    orig = bu.compile_bir_kernel

    def patched(bir_json: bytes, tmpdir, neff_name: str = "file.neff"):
        return orig(_legalize_single_wait(bir_json), tmpdir, neff_name)

    bu.compile_bir_kernel = patched
    b2j.compile_bir_kernel = patched

    bu._single_wait_shim = True


def _install_ntff_hook_shim():
    """antenv.axon_hooks is missing from this image; run_bass_kernel_spmd's
    trace path (BASS_TRACE=1) imports it.  Provide the module, wired to the
    same ctypes NTFF hook trn_boot would have registered."""
    import sys
    import types

    if "antenv.axon_hooks" in sys.modules:
        return
    hook = None
    try:
        import trn_agent_boot.trn_boot as trn_boot

        hook = trn_boot._ntff_profile_via_ctypes("/opt/axon/libaxon_pjrt.so")
    except Exception:
        pass
    mod = types.ModuleType("antenv.axon_hooks")
    mod._hook = hook
    mod.get_axon_ntff_profile_hook = lambda: mod._hook
    mod.set_axon_ntff_profile_hook = lambda h: setattr(mod, "_hook", h)
    sys.modules["antenv.axon_hooks"] = mod


_install_walrus_shim()
_install_ntff_hook_shim()


def _build(w0: float, w1: float, b0: float, b1: float) -> bass.Bass:
    nc = bass.Bass("TRN2", target_bir_lowering=False, debug=False,
                   num_devices=NCORES, num_swdge_queues=4)
    x = nc.dram_tensor("x", [B, D], F32, kind="ExternalInput").ap()
    out = nc.dram_tensor("out", [BC, B, 2], F32, kind="ExternalOutput").ap()
    ident_d = nc.inline_tensor(np.eye(P, dtype=np.float16), "ident")

    with tile.TileContext(nc) as tc, ExitStack() as ctx:
        xpool = ctx.enter_context(tc.tile_pool(name="xin", bufs=10))
        sqpool = ctx.enter_context(tc.tile_pool(name="sq", bufs=4))
        stat = ctx.enter_context(tc.tile_pool(name="stat", bufs=6))
        fpool = ctx.enter_context(tc.tile_pool(name="xn16", bufs=6))
        tpsum = ctx.enter_context(tc.tile_pool(name="tpsum", bufs=3, space="PSUM"))
        spsum = ctx.enter_context(tc.tile_pool(name="spsum", bufs=2, space="PSUM"))
        opool = ctx.enter_context(tc.tile_pool(name="outt", bufs=4))
        wpsum = ctx.enter_context(tc.tile_pool(name="wpsum", bufs=1, space="PSUM"))
        big = ctx.enter_context(tc.tile_pool(name="big", bufs=1))

        ident = big.tile([P, P], F16, name="ident_sb")
        nc.sync.dma_start(ident, ident_d.ap())
        xnT = big.tile([P, KT, B], F16, name="xnT")   # normalized x, transposed

        # Dummy matmuls with no data deps: the scheduler runs them during the
        # DMA/vector-bound prep phase, keeping the PE busy so the HAM clock
        # gate reaches (and holds) the full 2.4 GHz.
        wsrc = big.tile([P, 512], F16, name="warm_src")
        nc.vector.memset(wsrc, 0)
        wps = wpsum.tile([P, 512], F32, name="warm_ps")

        def warm(n_mm):
            for w in range(n_mm):
                nc.tensor.matmul(wps, wsrc[:, 0:P], wsrc, start=True, stop=True)

        warm(16)

        def prep_block(n, pfx=""):
            """Prep column block n: cast-load 4 x row-tiles as fp16, row
            sumsq (fp32 accum), batched rsqrt, normalize, PE-transpose into
            xnT[:, :, n*512:(n+1)*512].  Engines rotated per tile.

            The reference clamps ||x|| at eps=1e-8, which for randn inputs
            can never bind; we rely on sumsq > 0."""
            ssb = stat.tile([P, 4], F32, tag="ssb", name=f"ssb{pfx}{n}")
            xts = []
            for j in range(4):
                t = 4 * n + j
                xt = xpool.tile([P, D], F16, tag="xt", name=f"xt{pfx}{t}")
                nc.gpsimd.dma_start(xt, x[t * P:(t + 1) * P, :])  # fp32->fp16
                xts.append(xt)
                sq = sqpool.tile([P, D], F16, tag="sq", name=f"sqt{pfx}{t}")
                acc = ssb[:, j:j + 1]
                if j % 2 == 0:
                    nc.scalar.activation(sq, xt, AF.Square, accum_out=acc)
                else:
                    nc.vector.scalar_tensor_tensor(
                        sq, xt, 1.0, xt,
                        op0=ALU.bypass, op1=ALU.mult, accum_out=acc,
                    )
            rinb = stat.tile([P, 4], F32, tag="rinb", name=f"rinb{pfx}{n}")
            nc.vector.reciprocal(rinb, ssb)
            rb = stat.tile([P, 4], F32, tag="rb", name=f"rb{pfx}{n}")
            nc.scalar.sqrt(rb, rinb)                  # rsqrt(sumsq)
            for j in range(4):
                t = 4 * n + j
                r = rb[:, j:j + 1]
                xn = fpool.tile([P, D], F16, tag="xn", name=f"xn{pfx}{t}")
                if j % 2 == 0:
                    nc.gpsimd.tensor_scalar_mul(xn, xts[j], r)
                else:
                    nc.scalar.activation(xn, xts[j], AF.Copy, scale=r)
                pt = tpsum.tile([P, D], F16, tag="pt", name=f"pt{pfx}{t}")
                for k in range(KT):
                    nc.tensor.transpose(pt[:, k * P:(k + 1) * P],
                                        xn[:, k * P:(k + 1) * P], ident)
                # one batched PSUM->SBUF copy for all 6 k-slices of this tile
                # (GPSIMD cannot read PSUM on trn2: ACT/DVE only)
                ptv = pt.rearrange("p (k c) -> p k c", k=KT)
                dd = xnT[:, :, t * P:(t + 1) * P]
                if j % 2 == 0:
                    nc.scalar.copy(dd, ptv)
                else:
                    nc.vector.tensor_copy(dd, ptv)

        prep_block(0)          # own rows: stationary for every matmul
        warm(8)
        prep_block(1)

        for bp in range(4):            # block pairs (2*bp, 2*bp+1)
            if bp < 3:
                prep_block(2 * bp + 2)
                prep_block(2 * bp + 3)
            elif bp == 3:
                warm(4)
            for m in range(4):
                ps = spsum.tile([P, 1024], F32, tag="ps", name=f"ps{bp}_{m}")
                for jj in range(2):
                    n = 2 * bp + jj
                    for k in range(KT):
                        nc.tensor.matmul(
                            ps[:, jj * 512:(jj + 1) * 512],
                            xnT[:, k, m * P:(m + 1) * P],
                            xnT[:, k, n * 512:(n + 1) * 512],
                            start=(k == 0), stop=(k == KT - 1),
                        )
                ot = opool.tile([P, 1024, 2], F16, tag="ot", name=f"ot{bp}_{m}")
                nc.scalar.activation(ot[:, :, 0:1], ps, AF.Copy,
                                     bias=b0, scale=w0)
                nc.vector.tensor_scalar(
                    ot[:, :, 1:2], ps, w1, b1, op0=ALU.mult, op1=ALU.add
                )
                # SWDGE cast fp16 -> fp32 on the way out
                nc.gpsimd.dma_start(
                    out[m * P:(m + 1) * P, bp * 1024:(bp + 1) * 1024, :], ot
                )
    return nc


def kernel(x, fc_w, fc_b):
    global LAST_RESULTS
    x = np.ascontiguousarray(np.asarray(x, dtype=np.float32))
    fc_w = np.asarray(fc_w, dtype=np.float32)
    fc_b = np.asarray(fc_b, dtype=np.float32)
    nc = _build(float(fc_w[0, 0]), float(fc_w[1, 0]),
                float(fc_b[0]), float(fc_b[1]))
    # core c gets x rotated so its own rows come first (compile-time offsets)
    in_maps = [
        {"x": np.ascontiguousarray(np.roll(x, -c * BC, axis=0))}
        for c in range(NCORES)
    ]
    res = run_bass_kernel_spmd(nc, in_maps, core_ids=list(range(NCORES)))
    LAST_RESULTS = res
    # un-rotate each core's output columns back to global order
    return np.concatenate(
        [np.roll(res.results[c]["out"], c * BC, axis=1) for c in range(NCORES)],
        axis=0,
    )


# revision 10
# speedup vs baseline: 2.5142x; 2.5142x over previous
"""Pairwise cosine-similarity (normalize -> x @ x.T) + Linear(1,2) affine, on 8 trn2 cores.

Data-parallel over rows of x; each core computes a [512, 4096] slice of the
similarity matrix and writes [512, 4096, 2] fp32 output.

v2 design (vs the first working version):
  - Host-side rotation: core c receives x rolled so its own 512 rows come
    FIRST.  The kernel is pure SPMD with compile-time offsets (no per-core
    branching), stationary operand = xnT block 0.  The host un-rotates the
    output columns (np.roll on axis 1) after the run.
  - fp32 -> fp16 cast during the input DMA (SWDGE): halves SBUF-side input
    traffic and doubles effective vector-engine throughput for the whole
    prep pipeline (sumsq, normalize, transpose copies all fp16).
  - Row norms: fp16 squares accumulated in fp32 (accum_out), batched
    reciprocal+sqrt per 4-tile group, normalize fused with per-row scale.
  - PE transposes (fp16, 128x128) into 1-bank PSUM tiles; batched
    PSUM->SBUF copies build xnT [128, 6, 4096] fp16.
  - Matmul units: [128, 1024] fp32 PSUM (2 banks) = 12 fp16 matmuls
    (2 column blocks x 6 k-tiles).  Epilogue: 2 wide ops per unit
    (ACT ch0 / DVE ch1) writing interleaved fp16 [128, 1024, 2]; the
    output DMA casts fp16 -> fp32 in the SWDGE datapath.
  - Warm-up matmuls keep the PE HAM clock at 2.4 GHz through the prep phase.

Numerics: fp16 inputs/outputs with fp32 accumulation everywhere
(PE PSUM + activation internals).  Measured rel err ~1.5e-4 vs the fp32
reference (tolerance 2e-2).

This file monkeypatches two toolchain gaps at import: walrus here only
accepts one sync-wait per instruction (Tile emits several), and the
axon NTFF profile hook module may be absent when BASS_TRACE=1.
"""

import numpy as np
from contextlib import ExitStack

import concourse.bass as bass
import concourse.tile as tile
from concourse import mybir
from concourse.bass_utils import run_bass_kernel_spmd

B, D, NCORES = 4096, 768, 8
BC = B // NCORES          # 512 rows per core
P = 128                   # partitions
KT = D // P               # 6 contraction tiles
NB = B // 512             # 8 column blocks of 512
F16 = mybir.dt.float16
F32 = mybir.dt.float32
AF = mybir.ActivationFunctionType
ALU = mybir.AluOpType

LAST_RESULTS = None       # test harness peeks at exec_time_ns here


def _legalize_single_wait(bir_bytes: bytes) -> bytes:
    """This container's walrus accepts at most ONE sync wait per instruction,
    while Tile attaches several. Split extras into standalone EventSemaphore
    instructions inserted just before the owner (same engine stream, so the
    sequencer stalls at the same program point; schedule order is a global
    topological order, so earlier stalls cannot deadlock)."""
    import json

    d = json.loads(bir_bytes)
    n_split = 0
    for f in d.get("functions", []):
        for bb in f.get("blocks", []):
            insts = bb.get("instructions", [])
            out = []
            for ins in insts:
                si = ins.get("sync_info") or {}
                waits = si.get("on_wait") or []
                if len(waits) > 1:
                    keep = waits[-1]
                    for i, w in enumerate(waits[:-1]):
                        n_split += 1
                        out.append({
                            "debug": ins.get("debug", 0),
                            "engine": ins["engine"],
                            "ins": [],
                            "name": f"{ins['name']}__w{i}",
                            "opcode": "EventSemaphore",
                            "outs": [],
                            "sync_info": {"on_update": [], "on_wait": [w]},
                        })
                    si["on_wait"] = [keep]
                out.append(ins)
            bb["instructions"] = out
    return json.dumps(d).encode()


def _install_walrus_shim():
    """Route every BIR->NEFF compile through the single-wait legalizer."""
    import concourse.bass2jax as b2j
    import concourse.bass_utils as bu

    if getattr(bu, "_single_wait_shim", False):
        return
    orig = bu.compile_bir_kernel

    def patched(bir_json: bytes, tmpdir, neff_name: str = "file.neff"):
        return orig(_legalize_single_wait(bir_json), tmpdir, neff_name)

    bu.compile_bir_kernel = patched
    b2j.compile_bir_kernel = patched

    bu._single_wait_shim = True


def _install_ntff_hook_shim():
    """antenv.axon_hooks is missing from this image; run_bass_kernel_spmd's
    trace path (BASS_TRACE=1) imports it.  Provide the module, wired to the
    same ctypes NTFF hook trn_boot would have registered."""
    import sys
    import types

    if "antenv.axon_hooks" in sys.modules:
        return
    hook = None
    try:
        import trn_agent_boot.trn_boot as trn_boot

        hook = trn_boot._ntff_profile_via_ctypes("/opt/axon/libaxon_pjrt.so")
    except Exception:
        pass
    mod = types.ModuleType("antenv.axon_hooks")
    mod._hook = hook
    mod.get_axon_ntff_profile_hook = lambda: mod._hook
    mod.set_axon_ntff_profile_hook = lambda h: setattr(mod, "_hook", h)
    sys.modules["antenv.axon_hooks"] = mod


_install_walrus_shim()
_install_ntff_hook_shim()


def _build(w0: float, w1: float, b0: float, b1: float) -> bass.Bass:
    nc = bass.Bass("TRN2", target_bir_lowering=False, debug=False,
                   num_devices=NCORES, num_swdge_queues=4)
    x = nc.dram_tensor("x", [B, D], F32, kind="ExternalInput").ap()
    out = nc.dram_tensor("out", [BC, B, 2], F32, kind="ExternalOutput").ap()
    ident_d = nc.inline_tensor(np.eye(P, dtype=np.float16), "ident")

    with tile.TileContext(nc) as tc, ExitStack() as ctx:
        xpool = ctx.enter_context(tc.tile_pool(name="xin", bufs=10))
        sqpool = ctx.enter_context(tc.tile_pool(name="sq", bufs=4))
        stat = ctx.enter_context(tc.tile_pool(name="stat", bufs=6))
        fpool = ctx.enter_context(tc.tile_pool(name="xn16", bufs=6))
        tpsum = ctx.enter_context(tc.tile_pool(name="tpsum", bufs=3, space="PSUM"))
        spsum = ctx.enter_context(tc.tile_pool(name="spsum", bufs=2, space="PSUM"))
        opool = ctx.enter_context(tc.tile_pool(name="outt", bufs=4))
        wpsum = ctx.enter_context(tc.tile_pool(name="wpsum", bufs=1, space="PSUM"))
        big = ctx.enter_context(tc.tile_pool(name="big", bufs=1))

        ident = big.tile([P, P], F16, name="ident_sb")
        nc.sync.dma_start(ident, ident_d.ap())
        xnT = big.tile([P, KT, B], F16, name="xnT")   # normalized x, transposed

        # Dummy matmuls with no data deps: the scheduler runs them during the
        # DMA/vector-bound prep phase, keeping the PE busy so the HAM clock
        # gate reaches (and holds) the full 2.4 GHz.
        wsrc = big.tile([P, 512], F16, name="warm_src")
        nc.vector.memset(wsrc, 0)
        wps = wpsum.tile([P, 512], F32, name="warm_ps")

        def warm(n_mm):
            for w in range(n_mm):
                nc.tensor.matmul(wps, wsrc[:, 0:P], wsrc, start=True, stop=True)

        warm(16)

        def prep_block(n, pfx=""):
            """Prep column block n: cast-load 4 x row-tiles as fp16, row
            sumsq (fp32 accum), batched rsqrt, normalize, PE-transpose into
            xnT[:, :, n*512:(n+1)*512].  Engines rotated per tile.

            The reference clamps ||x|| at eps=1e-8, which for randn inputs
            can never bind; we rely on sumsq > 0."""
            ssb = stat.tile([P, 4], F32, tag="ssb", name=f"ssb{pfx}{n}")
            xts = []
            for j in range(4):
                t = 4 * n + j
                xt = xpool.tile([P, D], F16, tag="xt", name=f"xt{pfx}{t}")
                nc.gpsimd.dma_start(xt, x[t * P:(t + 1) * P, :])  # fp32->fp16
                xts.append(xt)
                sq = sqpool.tile([P, D], F16, tag="sq", name=f"sqt{pfx}{t}")
                acc = ssb[:, j:j + 1]
                # ACT Square for all: fp16 2-operand DVE ops (STT) measure
                # ~2.5us/tile vs ~0.65us here
                nc.scalar.activation(sq, xt, AF.Square, accum_out=acc)
            rinb = stat.tile([P, 4], F32, tag="rinb", name=f"rinb{pfx}{n}")
            nc.vector.reciprocal(rinb, ssb)
            rb = stat.tile([P, 4], F32, tag="rb", name=f"rb{pfx}{n}")
            nc.scalar.sqrt(rb, rinb)                  # rsqrt(sumsq)
            for j in range(4):
                t = 4 * n + j
                r = rb[:, j:j + 1]
                xn = fpool.tile([P, D], F16, tag="xn", name=f"xn{pfx}{t}")
                # DVE all normalizes (1-src fp16 runs at 2x rate; GpSimd
                # vector math is ~20x slower than DVE -- never use it)
                nc.vector.tensor_scalar_mul(xn, xts[j], r)
                pt = tpsum.tile([P, D], F16, tag="pt", name=f"pt{pfx}{t}")
                for k in range(KT):
                    nc.tensor.transpose(pt[:, k * P:(k + 1) * P],
                                        xn[:, k * P:(k + 1) * P], ident)
                # one batched PSUM->SBUF copy for all 6 k-slices of this tile
                # (GPSIMD cannot read PSUM on trn2: ACT/DVE only)
                ptv = pt.rearrange("p (k c) -> p k c", k=KT)
                dd = xnT[:, :, t * P:(t + 1) * P]
                if j % 2 == 0:
                    nc.scalar.copy(dd, ptv)
                else:
                    nc.vector.tensor_copy(dd, ptv)

        prep_block(0)          # own rows: stationary for every matmul
        warm(8)
        prep_block(1)

        for bp in range(4):            # block pairs (2*bp, 2*bp+1)
            if bp < 3:
                prep_block(2 * bp + 2)
                prep_block(2 * bp + 3)
            elif bp == 3:
                warm(4)
            for m in range(4):
                ps = spsum.tile([P, 1024], F32, tag="ps", name=f"ps{bp}_{m}")
                for jj in range(2):
                    n = 2 * bp + jj
                    for k in range(KT):
                        nc.tensor.matmul(
                            ps[:, jj * 512:(jj + 1) * 512],
                            xnT[:, k, m * P:(m + 1) * P],
                            xnT[:, k, n * 512:(n + 1) * 512],
                            start=(k == 0), stop=(k == KT - 1),
                        )
                ot = opool.tile([P, 1024, 2], F32, tag="ot", name=f"ot{bp}_{m}")
                nc.scalar.activation(ot[:, :, 0:1], ps, AF.Copy,
                                     bias=b0, scale=w0)
                nc.vector.tensor_scalar(
                    ot[:, :, 1:2], ps, w1, b1, op0=ALU.mult, op1=ALU.add
                )
                # HWDGE (sync is otherwise idle; keeps descgen off gpsimd,
                # which owns the 32 cast-loads)
                nc.sync.dma_start(
                    out[m * P:(m + 1) * P, bp * 1024:(bp + 1) * 1024, :], ot
                )
    return nc


def kernel(x, fc_w, fc_b):
    global LAST_RESULTS
    x = np.ascontiguousarray(np.asarray(x, dtype=np.float32))
    fc_w = np.asarray(fc_w, dtype=np.float32)
    fc_b = np.asarray(fc_b, dtype=np.float32)
    nc = _build(float(fc_w[0, 0]), float(fc_w[1, 0]),
                float(fc_b[0]), float(fc_b[1]))
    # core c gets x rotated so its own rows come first (compile-time offsets)
    in_maps = [
        {"x": np.ascontiguousarray(np.roll(x, -c * BC, axis=0))}
        for c in range(NCORES)
    ]
    res = run_bass_kernel_spmd(nc, in_maps, core_ids=list(range(NCORES)))
    LAST_RESULTS = res
    # un-rotate each core's output columns back to global order
    return np.concatenate(
        [np.roll(res.results[c]["out"], c * BC, axis=1) for c in range(NCORES)],
        axis=0,
    )
